# revision 39
# baseline (speedup 1.0000x reference)
"""Trainium2 Bass kernel for sliding-window multi-head attention.

Problem (nn_MultiHeadAttention_74285754352148):
  B=2, S=2048, D=1024, H=16, HD=64, WINDOW=512 (causal, j in [i-256, i]),
  RoPE theta=10000, out = softmax(mask(QK^T)/8) V @ Wo + bo.

Sharding: batch x sequence across 8 cores (core c: batch c//4, tokens
[512*(c%4), 512*(c%4)+512)). Each core recomputes K/V for a 256-token halo;
no collectives. Host pre-transposes X and pre-rounds matmul operands to
fp32r (8-bit exp / 11-bit mantissa -> full-rate PE).

Key structure per core (all layouts chosen to avoid on-chip transposes of
activations except ctx):
  qropeT[d, tok]  = RoPE(Wq^T @ X^T)   (dim-major)
  kropeT[d, tok]  = RoPE(Wk^T @ X^T)
  V[tok, d]       = X @ Wv             (token-major, +ones column per head)
  scoresT[k, q]   = kropeT^T-slices @ qropeT-slices  (keys on partitions)
  pT = exp(scoresT/8); band masks applied in-place via gpsimd affine_select
  ctx[q, 65]      = pT-chunks^T @ V_aug  (col 64 = softmax denominator)
  ctx normalized via fused tensor_scalar copy, PE-transposed to ctxT
  outT[d, tok]    = Wo^T-slices @ ctxT  (+bias), host transposes back
"""

import numpy as np

import concourse.bass as bass
import concourse.bacc as bacc
import concourse.mybir as mybir
from concourse.tile import TileContext
from concourse.bass import ts
from concourse.bass_utils import run_bass_kernel_spmd

F32 = mybir.dt.float32
F32R = mybir.dt.float32r
BF16 = mybir.dt.bfloat16

B, S, D = 2, 2048, 1024
H, HD = 16, 64
HALF_W = 256          # window // 2: query i attends keys [i-256, i]
TC = 512              # tokens per core
TH = TC + HALF_W      # tokens incl halo = 768
NQB = TC // 128       # query blocks per core = 4
NKC = 3               # key chunks per query block (384 = 3*128)
DC = D // 128         # 8 partition chunks of the model dim
NTC = TH // 128       # token chunks incl halo = 6
VW = HD + 1           # per-head V width incl ones column = 65
THETA = 10000.0


def round_fp32r(x: np.ndarray) -> np.ndarray:
    """Round-to-nearest (ties-to-even-ish) to fp32r: low 12 mantissa bits zero."""
    b = np.ascontiguousarray(x, dtype=np.float32).view(np.uint32)
    out = (b + np.uint32(0x7FF) + ((b >> np.uint32(12)) & np.uint32(1))) & np.uint32(
        0xFFFFF000
    )
    return out.view(np.float32)


SHIFT_MODE = "pe"  # pe | sync | scalar | none (timing experiment only)


def build_nc(loop_repeat=None):
    nc = bacc.Bacc(None, target_bir_lowering=False)

    xt = nc.dram_tensor("xt", [D, TH], F32R, kind="ExternalInput")
    wq = nc.dram_tensor("wq", [128, DC * D], F32R, kind="ExternalInput")
    wk = nc.dram_tensor("wk", [128, DC * D], F32R, kind="ExternalInput")
    wv = nc.dram_tensor("wv", [D, D], F32R, kind="ExternalInput")
    wo = nc.dram_tensor("wo", [128, DC * D], F32R, kind="ExternalInput")
    bo = nc.dram_tensor("bo", [128, DC], F32, kind="ExternalInput")
    cosq = nc.dram_tensor("cosq", [128, TC], F32, kind="ExternalInput")
    sinq2 = nc.dram_tensor("sinq2", [128, TC], F32, kind="ExternalInput")
    cosk = nc.dram_tensor("cosk", [128, TH], F32, kind="ExternalInput")
    sink2 = nc.dram_tensor("sink2", [128, TH], F32, kind="ExternalInput")
    corr = nc.dram_tensor("corr", [128, NQB], F32, kind="ExternalInput")
    ident_d = nc.dram_tensor("ident", [128, 128], F32R, kind="ExternalInput")
    perm_d = nc.dram_tensor("perm32", [128, 128], F32R, kind="ExternalInput")
    band_d = nc.dram_tensor("band", [128, 1024], F32R, kind="ExternalInput")
    outT = nc.dram_tensor("outT", [D, TC], F32, kind="ExternalOutput")

    with TileContext(nc) as tc:
        with (
            tc.tile_pool(name="qkp", bufs=1) as qkp,
            tc.tile_pool(name="vp", bufs=1) as vp,
            tc.tile_pool(name="tbl", bufs=1) as tbl,
            tc.tile_pool(name="sm", bufs=8) as sm,
            tc.tile_pool(name="wpool", bufs=3) as wpool,
            tc.tile_pool(name="xtp", bufs=1) as xtp,
            tc.tile_pool(name="uwp", bufs=2) as uwp,
            tc.tile_pool(name="pp", bufs=3) as pp,
            tc.tile_pool(name="cxp", bufs=2) as cxp,
            tc.tile_pool(name="cxtp", bufs=1) as cxtp,
            tc.tile_pool(name="op", bufs=3) as op,
            tc.tile_pool(name="proj_ps", bufs=2, space="PSUM") as proj_ps,
            tc.tile_pool(name="sc_ps", bufs=2, space="PSUM") as sc_ps,
            tc.tile_pool(name="ctx_ps", bufs=2, space="PSUM") as ctx_ps,
        ):
            # ---- constant/table loads ----
            cosq_sb = tbl.tile([128, TC], F32)
            sinq2_sb = tbl.tile([128, TC], F32)
            cosk_sb = tbl.tile([128, TH], F32)
            sink2_sb = tbl.tile([128, TH], F32)
            corr_sb = tbl.tile([128, NQB], F32)
            bo_sb = tbl.tile([128, DC], F32)
            for t_dram, t_sb in [
                (cosq, cosq_sb),
                (sinq2, sinq2_sb),
                (cosk, cosk_sb),
                (sink2, sink2_sb),
                (corr, corr_sb),
                (bo, bo_sb),
            ]:
                nc.sync.dma_start(out=t_sb, in_=t_dram[:, :])
            ident = tbl.tile([128, 128], F32R)
            nc.sync.dma_start(out=ident, in_=ident_d[:, :])
            perm32 = tbl.tile([128, 128], F32R)
            nc.sync.dma_start(out=perm32, in_=perm_d[:, :])
            band_sb = tbl.tile([128, 1024], F32R)
            nc.sync.dma_start(out=band_sb, in_=band_d[:, :])
            ones16 = tbl.tile([128, H], F32)
            nc.vector.memset(ones16, 1.0)

            def body():
                # ---- input loads: query cols first so Q-proj starts early
                xt_sb = xtp.tile([128, DC, TH], F32R)
                for k in range(DC):
                    nc.sync.dma_start(
                        out=xt_sb[:, k, HALF_W:TH], in_=xt[ts(k, 128), HALF_W:TH]
                    )

                def load_w_blocked(w_dram, nm):
                    """dc-blocked: host layout [p, dc, k, c]; access (k, dc)."""
                    halves = []
                    for hh in range(2):
                        w_sb = wpool.tile(
                            [128, DC // 2, DC, 128], F32R, tag="w", name=f"w_{nm}{hh}"
                        )
                        for dcl in range(DC // 2):
                            off = (hh * 4 + dcl) * D
                            nc.sync.dma_start(
                                out=w_sb[:, dcl], in_=w_dram[:, off : off + D]
                            )
                        halves.append(w_sb)
                    return lambda k, dc: halves[dc // 4][:, dc % 4, k]

                def load_w(w_dram, nm):
                    """Two half-matrix tiles [128, 4, 1024] sharing 3 slots."""
                    halves = []
                    for hh in range(2):
                        w_sb = wpool.tile(
                            [128, DC // 2, D], F32R, tag="w", name=f"w_{nm}{hh}"
                        )
                        for k in range(DC // 2):
                            nc.sync.dma_start(
                                out=w_sb[:, k], in_=w_dram[ts(hh * 4 + k, 128), :]
                            )
                        halves.append(w_sb)
                    return lambda k: halves[k // 4][:, k % 4]

                wq_at = load_w_blocked(wq, "q")
                for k in range(DC):
                    nc.sync.dma_start(
                        out=xt_sb[:, k, 0:HALF_W], in_=xt[ts(k, 128), 0:HALF_W]
                    )
                wk_at = load_w_blocked(wk, "k")

                qrope = qkp.tile([128, DC, TC], F32R)
                krope = qkp.tile([128, DC, TH], F32R)

                def rope_epilogue(ps, cos_sb, sin2_sb, cslc, out_ap):
                    """out = ps*cos + shift32(ps*sin2); ps is PSUM [128, n]."""
                    n = ps.shape[-1]
                    u = uwp.tile([128, n], F32, tag="u")
                    nc.vector.scalar_tensor_tensor(
                        out=u, in0=ps, scalar=1.0, in1=cos_sb[:, cslc],
                        op0=mybir.AluOpType.bypass, op1=mybir.AluOpType.mult,
                    )
                    if SHIFT_MODE == "pe":
                        w = uwp.tile([128, n], F32R, tag="w")
                        nc.vector.scalar_tensor_tensor(
                            out=w, in0=ps, scalar=1.0, in1=sin2_sb[:, cslc],
                            op0=mybir.AluOpType.bypass, op1=mybir.AluOpType.mult,
                        )
                        ws_ps = sc_ps.tile([128, n], F32, tag="sc")
                        nc.tensor.matmul(ws_ps, perm32, w, start=True, stop=True)
                        nc.vector.tensor_add(out_ap, ws_ps, u)
                        return
                    w = uwp.tile([128, n], F32, tag="w")
                    ws = uwp.tile([128, n], F32, tag="ws")
                    nc.vector.scalar_tensor_tensor(
                        out=w, in0=ps, scalar=1.0, in1=sin2_sb[:, cslc],
                        op0=mybir.AluOpType.bypass, op1=mybir.AluOpType.mult,
                    )
                    if SHIFT_MODE == "none":
                        nc.vector.tensor_add(out_ap, u, w)
                        return
                    eng = nc.sync if SHIFT_MODE == "sync" else nc.scalar
                    for a in range(2):
                        eng.dma_start(out=ws[a * 64 : a * 64 + 32], in_=w[a * 64 + 32 : a * 64 + 64])
                        eng.dma_start(out=ws[a * 64 + 32 : a * 64 + 64], in_=w[a * 64 : a * 64 + 32])
                    nc.vector.tensor_add(out_ap, u, ws)

                # ---- Q^T projection + RoPE (dim-major) ----
                for dc in range(DC):
                    ps = proj_ps.tile([128, TC], F32, tag="proj")
                    for k in range(DC):
                        nc.tensor.matmul(
                            ps, wq_at(k, dc), xt_sb[:, k, HALF_W:TH],
                            start=(k == 0), stop=(k == DC - 1),
                        )
                    rope_epilogue(ps, cosq_sb, sinq2_sb, slice(0, TC), qrope[:, dc])

                # ---- K^T projection + RoPE, two 384-col halves ----
                for dc in range(DC):
                    for half in range(2):
                        cs = slice(half * 384, half * 384 + 384)
                        ps = proj_ps.tile([128, 384], F32, tag="proj")
                        for k in range(DC):
                            nc.tensor.matmul(
                                ps, wk_at(k, dc), xt_sb[:, k, cs],
                                start=(k == 0), stop=(k == DC - 1),
                            )
                        rope_epilogue(ps, cosk_sb, sink2_sb, cs, krope[:, dc, cs])

                wv_at = load_w(wv, "v")

                # ---- V projection (token-major, 65-wide per-head groups) ----
                v_sb = vp.tile([128, NTC, H * VW], BF16)
                for tcn in range(NTC):
                    v_grp = v_sb[:, tcn].rearrange("p (h c) -> p h c", c=VW)
                    nc.vector.tensor_copy(
                        v_grp[:, :, HD:VW], ones16.rearrange("p (a b) -> p a b", b=1)
                    )
                    for half in range(2):
                        ps = proj_ps.tile([128, 512], F32, tag="proj")
                        for k in range(DC):
                            nc.tensor.matmul(
                                ps, xt_sb[:, k, ts(tcn, 128)], wv_at(k)[:, ts(half, 512)],
                                start=(k == 0), stop=(k == DC - 1),
                            )
                        nc.scalar.copy(
                            out=v_grp[:, half * 8 : half * 8 + 8, 0:HD],
                            in_=ps.rearrange("p (h c) -> p h c", c=HD),
                        )

                wo_at = load_w_blocked(wo, "o")

                # ---- attention: per 256-query pair-block, per head ----
                # scoresT chunks kc=0..3 cover keys [qs-256, qs+256) for the
                # 256-query block at qs; each chunk needs exactly one affine
                # band mask (x = key partition, y = query free coord).
                ctxT = cxtp.tile([128, DC, TC], F32R)
                AFF = [
                    (1, 0, [[-1, 256]]),     # x - y >= 0
                    (1, 128, [[-1, 256]]),   # x - y + 128 >= 0
                    (-1, 0, [[1, 256]]),     # y - x >= 0
                    (-1, -128, [[1, 256]]),  # y - x - 128 >= 0
                ]
                for qp in range(NQB // 2):
                    ctx_hf = [
                        cxp.tile([128, D], F32R, tag="ctx", name=f"ctx_{qp}_{i}")
                        for i in range(2)
                    ]
                    for h2 in range(H // 2):
                        dc = h2
                        # scores for both heads back-to-back: the two heads
                        # sit in PE row groups 0-63 / 64-127 and co-run
                        psS_all = []
                        for hp_i in range(2):
                            hp = 64 * hp_i
                            psS = sc_ps.tile(
                                [128, 4, 256], F32, tag="sc",
                                name=f"psS_{qp}_{h2}_{hp_i}",
                            )
                            for kc in range(4):
                                kcol = qp * 256 + kc * 128
                                nc.tensor.matmul(
                                    psS[:, kc],
                                    krope[hp : hp + 64, dc, kcol : kcol + 128],
                                    qrope[hp : hp + 64, dc, qp * 256 : qp * 256 + 256],
                                    start=(kc % 2 == 0), stop=False,
                                )
                                if kc % 2 == 1:
                                    bank = kc // 2
                                    nc.tensor.matmul(
                                        psS[:, 2 * bank : 2 * bank + 2].rearrange(
                                            "p a b -> p (a b)"
                                        ),
                                        ident,
                                        band_sb[:, 512 * bank : 512 * bank + 512],
                                        start=False, stop=True,
                                    )
                            psS_all.append(psS)
                        pTs_h = []
                        for hp_i in range(2):
                            pT = pp.tile(
                                [128, 4, 256], BF16, tag="pT",
                                name=f"pT_{qp}_{h2}_{hp_i}",
                            )
                            nc.scalar.activation(
                                pT, psS_all[hp_i],
                                mybir.ActivationFunctionType.Exp, scale=0.125,
                            )
                            pTs_h.append(pT)
                        for hp_i in range(2):
                            h = 2 * h2 + hp_i
                            pT = pTs_h[hp_i]
                            for hf in range(2):  # query halves use chunks hf..hf+2
                                psC = ctx_ps.tile([128, VW], F32, tag="ctx")
                                for i, kc in enumerate(range(hf, hf + NKC)):
                                    nc.tensor.matmul(
                                        psC,
                                        pT[:, kc, hf * 128 : hf * 128 + 128],
                                        v_sb[:, qp * 2 + kc, h * VW : h * VW + VW],
                                        start=(i == 0), stop=(i == NKC - 1),
                                    )
                                qb = qp * 2 + hf
                                d2 = sm.tile([128, 1], F32, tag="d2")
                                rinv = sm.tile([128, 1], F32, tag="rinv")
                                nc.vector.tensor_tensor(
                                    out=d2, in0=psC[:, HD : HD + 1],
                                    in1=corr_sb[:, qb : qb + 1],
                                    op=mybir.AluOpType.subtract,
                                )
                                nc.vector.reciprocal(rinv, d2)
                                nc.vector.tensor_scalar_mul(
                                    ctx_hf[hf][:, h * HD : h * HD + HD], psC[:, 0:HD], rinv
                                )
                    # ctx -> ctxT (PE transpose) for both query blocks
                    for hf in range(2):
                        qb = qp * 2 + hf
                        for dc in range(DC):
                            psT = sc_ps.tile([128, 128], F32R, tag="sc", name=f"psT_{qp}_{hf}_{dc}")
                            nc.tensor.transpose(psT, ctx_hf[hf][:, ts(dc, 128)], ident)
                            if dc % 2 == 0:
                                nc.vector.tensor_copy(ctxT[:, dc, ts(qb, 128)], psT)
                            else:
                                nc.scalar.copy(out=ctxT[:, dc, ts(qb, 128)], in_=psT)

                # ---- Wo projection, split by query pair-block for overlap
                for qpo in range(2):
                    cs = slice(qpo * 256, qpo * 256 + 256)
                    for dco in range(DC):
                        ps = proj_ps.tile([128, 256], F32, tag="proj")
                        for k in range(DC):
                            nc.tensor.matmul(
                                ps, wo_at(k, dco), ctxT[:, k, cs],
                                start=(k == 0), stop=(k == DC - 1),
                            )
                        o_sb = op.tile([128, 256], F32, tag="o")
                        nc.scalar.activation(
                            o_sb, ps, mybir.ActivationFunctionType.Identity,
                            bias=bo_sb[:, dco : dco + 1], scale=1.0,
                        )
                        nc.sync.dma_start(out=outT[ts(dco, 128), cs], in_=o_sb)

            if loop_repeat is None:
                body()
            else:
                with tc.For_i(0, loop_repeat, 1):
                    body()

    nc.compile()
    return nc


_NC_CACHE = None


def _get_nc():
    global _NC_CACHE
    if _NC_CACHE is None:
        _NC_CACHE = build_nc()
    return _NC_CACHE


def _host_tables():
    """RoPE cos/sin tables, dim-major, tiled to 128 partitions (2 heads)."""
    inv_freq = 1.0 / (THETA ** (np.arange(0, HD, 2, dtype=np.float32) / HD))  # [32]
    ifq64 = np.concatenate([inv_freq, inv_freq])  # dim d uses inv_freq[d % 32]

    def tables(positions):
        ang = ifq64[:, None] * positions[None, :].astype(np.float32)  # [64, n]
        cos = np.cos(ang).astype(np.float32)
        sin = np.sin(ang).astype(np.float32)
        sin2 = np.concatenate([sin[:32], -sin[32:]], axis=0)  # sign flip 2nd half
        return np.tile(cos, (2, 1)), np.tile(sin2, (2, 1))

    return tables


def _dc_block(w):
    """[D, D] -> [128, DC*D] with per-partition layout [dc, k, c]."""
    return np.ascontiguousarray(
        np.asarray(w, dtype=np.float32)
        .reshape(DC, 128, DC, 128)
        .transpose(1, 2, 0, 3)
        .reshape(128, DC * D)
    )


def _make_band_qp():
    """[128, 4, 256] qp-major band-mask: 0 where valid, -1e8 where not."""
    band = np.zeros((128, 4, 256), dtype=np.float32)
    x = np.arange(128)[:, None]
    y = np.arange(256)[None, :]
    for kc in range(4):
        xg = kc * 128 + x - 256
        valid = (y - xg >= 0) & (y - xg <= 256)
        band[:, kc] = np.where(valid, 0.0, -1.0e8)
    return band.reshape(128, 1024)


def prep_in_maps(input_sequence, Wq, Wk, Wv, Wo, bo):
    x = np.asarray(input_sequence, dtype=np.float32)
    wq_r = round_fp32r(_dc_block(Wq))
    wk_r = round_fp32r(_dc_block(Wk))
    wv_r = round_fp32r(np.asarray(Wv))
    wo_r = round_fp32r(_dc_block(Wo))
    bo_t = np.asarray(bo, dtype=np.float32).reshape(DC, 128).T.copy()

    tables = _host_tables()
    in_maps = []
    for c in range(8):
        b, t = c // 4, c % 4
        start = t * TC
        lo = start - HALF_W
        xt = np.zeros((D, TH), dtype=np.float32)
        vs = max(0, lo)
        xt[:, vs - lo : TH] = x[b, vs : start + TC, :].T
        cosq_t, sinq2_t = tables(np.arange(start, start + TC))
        cosk_t, sink2_t = tables(np.arange(lo, start + TC))
        qpos = np.arange(start, start + TC)
        corr = np.maximum(0, HALF_W - qpos).astype(np.float32).reshape(NQB, 128).T.copy()
        in_maps.append(
            {
                "xt": round_fp32r(xt),
                "wq": wq_r, "wk": wk_r, "wv": wv_r, "wo": wo_r,
                "bo": bo_t,
                "cosq": cosq_t, "sinq2": sinq2_t,
                "cosk": cosk_t, "sink2": sink2_t,
                "corr": corr,
                "ident": np.eye(128, dtype=np.float32),
                "perm32": np.eye(128, dtype=np.float32)[
                    [p ^ 32 for p in range(128)]
                ].copy(),
                "band": _make_band_qp(),
            }
        )
    return in_maps


def kernel(input_sequence, Wq, Wk, Wv, Wo, bo):
    nc = _get_nc()
    in_maps = prep_in_maps(input_sequence, Wq, Wk, Wv, Wo, bo)
    res = run_bass_kernel_spmd(nc, in_maps, list(range(8)))
    out = np.empty((B, S, D), dtype=np.float32)
    for c in range(8):
        b, t = c // 4, c % 4
        out[b, t * TC : t * TC + TC, :] = res.results[c]["outT"].T
    return out



# revision 40
# speedup vs baseline: 1.0646x; 1.0646x over previous
"""Trainium2 Bass kernel for sliding-window multi-head attention.

Problem (nn_MultiHeadAttention_74285754352148):
  B=2, S=2048, D=1024, H=16, HD=64, WINDOW=512 (causal, j in [i-256, i]),
  RoPE theta=10000, out = softmax(mask(QK^T)/8) V @ Wo + bo.

Sharding: batch x sequence across 8 cores (core c: batch c//4, tokens
[512*(c%4), 512*(c%4)+512)). Each core recomputes K/V for a 256-token halo;
no collectives. Host pre-transposes X and pre-rounds matmul operands to
fp32r (8-bit exp / 11-bit mantissa -> full-rate PE).

Key structure per core (all layouts chosen to avoid on-chip transposes of
activations except ctx):
  qropeT[d, tok]  = RoPE(Wq^T @ X^T)   (dim-major)
  kropeT[d, tok]  = RoPE(Wk^T @ X^T)
  V[tok, d]       = X @ Wv             (token-major, +ones column per head)
  scoresT[k, q]   = kropeT^T-slices @ qropeT-slices  (keys on partitions)
  pT = exp(scoresT/8); band masks applied in-place via gpsimd affine_select
  ctx[q, 65]      = pT-chunks^T @ V_aug  (col 64 = softmax denominator)
  ctx normalized via fused tensor_scalar copy, PE-transposed to ctxT
  outT[d, tok]    = Wo^T-slices @ ctxT  (+bias), host transposes back
"""

import numpy as np

import concourse.bass as bass
import concourse.bacc as bacc
import concourse.mybir as mybir
from concourse.tile import TileContext
from concourse.bass import ts
from concourse.bass_utils import run_bass_kernel_spmd

F32 = mybir.dt.float32
F32R = mybir.dt.float32r
BF16 = mybir.dt.bfloat16

B, S, D = 2, 2048, 1024
H, HD = 16, 64
HALF_W = 256          # window // 2: query i attends keys [i-256, i]
TC = 512              # tokens per core
TH = TC + HALF_W      # tokens incl halo = 768
NQB = TC // 128       # query blocks per core = 4
NKC = 3               # key chunks per query block (384 = 3*128)
DC = D // 128         # 8 partition chunks of the model dim
NTC = TH // 128       # token chunks incl halo = 6
VW = HD + 1           # per-head V width incl ones column = 65
THETA = 10000.0


def round_fp32r(x: np.ndarray) -> np.ndarray:
    """Round-to-nearest (ties-to-even-ish) to fp32r: low 12 mantissa bits zero."""
    b = np.ascontiguousarray(x, dtype=np.float32).view(np.uint32)
    out = (b + np.uint32(0x7FF) + ((b >> np.uint32(12)) & np.uint32(1))) & np.uint32(
        0xFFFFF000
    )
    return out.view(np.float32)


SHIFT_MODE = "pe"  # pe | sync | scalar | none (timing experiment only)


def build_nc(loop_repeat=None):
    nc = bacc.Bacc(None, target_bir_lowering=False)

    xt = nc.dram_tensor("xt", [D, TH], F32R, kind="ExternalInput")
    wq = nc.dram_tensor("wq", [128, DC * D], F32R, kind="ExternalInput")
    wk = nc.dram_tensor("wk", [128, DC * D], F32R, kind="ExternalInput")
    wv = nc.dram_tensor("wv", [D, D], F32R, kind="ExternalInput")
    wo = nc.dram_tensor("wo", [128, DC * D], F32R, kind="ExternalInput")
    bo = nc.dram_tensor("bo", [128, DC], F32, kind="ExternalInput")
    cosq = nc.dram_tensor("cosq", [128, TC], F32, kind="ExternalInput")
    sinq2 = nc.dram_tensor("sinq2", [128, TC], F32, kind="ExternalInput")
    cosk = nc.dram_tensor("cosk", [128, TH], F32, kind="ExternalInput")
    sink2 = nc.dram_tensor("sink2", [128, TH], F32, kind="ExternalInput")
    corr = nc.dram_tensor("corr", [128, NQB], F32, kind="ExternalInput")
    ident_d = nc.dram_tensor("ident", [128, 128], F32R, kind="ExternalInput")
    perm_d = nc.dram_tensor("perm32", [128, 128], F32R, kind="ExternalInput")
    outT = nc.dram_tensor("outT", [D, TC], F32, kind="ExternalOutput")

    with TileContext(nc) as tc:
        with (
            tc.tile_pool(name="qkp", bufs=1) as qkp,
            tc.tile_pool(name="vp", bufs=1) as vp,
            tc.tile_pool(name="tbl", bufs=1) as tbl,
            tc.tile_pool(name="sm", bufs=8) as sm,
            tc.tile_pool(name="wpool", bufs=3) as wpool,
            tc.tile_pool(name="xtp", bufs=1) as xtp,
            tc.tile_pool(name="uwp", bufs=2) as uwp,
            tc.tile_pool(name="pp", bufs=3) as pp,
            tc.tile_pool(name="cxp", bufs=2) as cxp,
            tc.tile_pool(name="cxtp", bufs=1) as cxtp,
            tc.tile_pool(name="op", bufs=3) as op,
            tc.tile_pool(name="proj_ps", bufs=2, space="PSUM") as proj_ps,
            tc.tile_pool(name="sc_ps", bufs=2, space="PSUM") as sc_ps,
            tc.tile_pool(name="ctx_ps", bufs=2, space="PSUM") as ctx_ps,
        ):
            # ---- constant/table loads ----
            cosq_sb = tbl.tile([128, TC], F32)
            sinq2_sb = tbl.tile([128, TC], F32)
            cosk_sb = tbl.tile([128, TH], F32)
            sink2_sb = tbl.tile([128, TH], F32)
            corr_sb = tbl.tile([128, NQB], F32)
            bo_sb = tbl.tile([128, DC], F32)
            for t_dram, t_sb in [
                (cosq, cosq_sb),
                (sinq2, sinq2_sb),
                (cosk, cosk_sb),
                (sink2, sink2_sb),
                (corr, corr_sb),
                (bo, bo_sb),
            ]:
                nc.sync.dma_start(out=t_sb, in_=t_dram[:, :])
            ident = tbl.tile([128, 128], F32R)
            nc.sync.dma_start(out=ident, in_=ident_d[:, :])
            perm32 = tbl.tile([128, 128], F32R)
            nc.sync.dma_start(out=perm32, in_=perm_d[:, :])
            ones16 = tbl.tile([128, H], F32)
            nc.vector.memset(ones16, 1.0)

            def body():
                # ---- input loads: query cols first so Q-proj starts early
                xt_sb = xtp.tile([128, DC, TH], F32R)
                for k in range(DC):
                    nc.sync.dma_start(
                        out=xt_sb[:, k, HALF_W:TH], in_=xt[ts(k, 128), HALF_W:TH]
                    )

                def load_w_blocked(w_dram, nm):
                    """dc-blocked: host layout [p, dc, k, c]; access (k, dc)."""
                    halves = []
                    for hh in range(2):
                        w_sb = wpool.tile(
                            [128, DC // 2, DC, 128], F32R, tag="w", name=f"w_{nm}{hh}"
                        )
                        for dcl in range(DC // 2):
                            off = (hh * 4 + dcl) * D
                            nc.sync.dma_start(
                                out=w_sb[:, dcl], in_=w_dram[:, off : off + D]
                            )
                        halves.append(w_sb)
                    return lambda k, dc: halves[dc // 4][:, dc % 4, k]

                def load_w(w_dram, nm):
                    """Two half-matrix tiles [128, 4, 1024] sharing 3 slots."""
                    halves = []
                    for hh in range(2):
                        w_sb = wpool.tile(
                            [128, DC // 2, D], F32R, tag="w", name=f"w_{nm}{hh}"
                        )
                        for k in range(DC // 2):
                            nc.sync.dma_start(
                                out=w_sb[:, k], in_=w_dram[ts(hh * 4 + k, 128), :]
                            )
                        halves.append(w_sb)
                    return lambda k: halves[k // 4][:, k % 4]

                wq_at = load_w_blocked(wq, "q")
                for k in range(DC):
                    nc.sync.dma_start(
                        out=xt_sb[:, k, 0:HALF_W], in_=xt[ts(k, 128), 0:HALF_W]
                    )
                wk_at = load_w_blocked(wk, "k")

                qrope = qkp.tile([128, DC, TC], F32R)
                krope = qkp.tile([128, DC, TH], F32R)

                def rope_epilogue(ps, cos_sb, sin2_sb, cslc, out_ap):
                    """out = ps*cos + shift32(ps*sin2); ps is PSUM [128, n]."""
                    n = ps.shape[-1]
                    u = uwp.tile([128, n], F32, tag="u")
                    nc.vector.scalar_tensor_tensor(
                        out=u, in0=ps, scalar=1.0, in1=cos_sb[:, cslc],
                        op0=mybir.AluOpType.bypass, op1=mybir.AluOpType.mult,
                    )
                    if SHIFT_MODE == "pe":
                        w = uwp.tile([128, n], F32R, tag="w")
                        nc.vector.scalar_tensor_tensor(
                            out=w, in0=ps, scalar=1.0, in1=sin2_sb[:, cslc],
                            op0=mybir.AluOpType.bypass, op1=mybir.AluOpType.mult,
                        )
                        ws_ps = sc_ps.tile([128, n], F32, tag="sc")
                        nc.tensor.matmul(ws_ps, perm32, w, start=True, stop=True)
                        nc.vector.tensor_add(out_ap, ws_ps, u)
                        return
                    w = uwp.tile([128, n], F32, tag="w")
                    ws = uwp.tile([128, n], F32, tag="ws")
                    nc.vector.scalar_tensor_tensor(
                        out=w, in0=ps, scalar=1.0, in1=sin2_sb[:, cslc],
                        op0=mybir.AluOpType.bypass, op1=mybir.AluOpType.mult,
                    )
                    if SHIFT_MODE == "none":
                        nc.vector.tensor_add(out_ap, u, w)
                        return
                    eng = nc.sync if SHIFT_MODE == "sync" else nc.scalar
                    for a in range(2):
                        eng.dma_start(out=ws[a * 64 : a * 64 + 32], in_=w[a * 64 + 32 : a * 64 + 64])
                        eng.dma_start(out=ws[a * 64 + 32 : a * 64 + 64], in_=w[a * 64 : a * 64 + 32])
                    nc.vector.tensor_add(out_ap, u, ws)

                # ---- Q^T projection + RoPE (dim-major) ----
                for dc in range(DC):
                    ps = proj_ps.tile([128, TC], F32, tag="proj")
                    for k in range(DC):
                        nc.tensor.matmul(
                            ps, wq_at(k, dc), xt_sb[:, k, HALF_W:TH],
                            start=(k == 0), stop=(k == DC - 1),
                        )
                    rope_epilogue(ps, cosq_sb, sinq2_sb, slice(0, TC), qrope[:, dc])

                # ---- K^T projection + RoPE, two 384-col halves ----
                for dc in range(DC):
                    for half in range(2):
                        cs = slice(half * 384, half * 384 + 384)
                        ps = proj_ps.tile([128, 384], F32, tag="proj")
                        for k in range(DC):
                            nc.tensor.matmul(
                                ps, wk_at(k, dc), xt_sb[:, k, cs],
                                start=(k == 0), stop=(k == DC - 1),
                            )
                        rope_epilogue(ps, cosk_sb, sink2_sb, cs, krope[:, dc, cs])

                wv_at = load_w(wv, "v")

                # ---- V projection (token-major, 65-wide per-head groups) ----
                v_sb = vp.tile([128, NTC, H * VW], BF16)
                for tcn in range(NTC):
                    v_grp = v_sb[:, tcn].rearrange("p (h c) -> p h c", c=VW)
                    nc.vector.tensor_copy(
                        v_grp[:, :, HD:VW], ones16.rearrange("p (a b) -> p a b", b=1)
                    )
                    for half in range(2):
                        ps = proj_ps.tile([128, 512], F32, tag="proj")
                        for k in range(DC):
                            nc.tensor.matmul(
                                ps, xt_sb[:, k, ts(tcn, 128)], wv_at(k)[:, ts(half, 512)],
                                start=(k == 0), stop=(k == DC - 1),
                            )
                        nc.scalar.copy(
                            out=v_grp[:, half * 8 : half * 8 + 8, 0:HD],
                            in_=ps.rearrange("p (h c) -> p h c", c=HD),
                        )

                wo_at = load_w_blocked(wo, "o")

                # ---- attention: per 256-query pair-block, per head ----
                # scoresT chunks kc=0..3 cover keys [qs-256, qs+256) for the
                # 256-query block at qs; each chunk needs exactly one affine
                # band mask (x = key partition, y = query free coord).
                ctxT = cxtp.tile([128, DC, TC], F32R)
                AFF = [
                    (1, 0, [[-1, 256]]),     # x - y >= 0
                    (1, 128, [[-1, 256]]),   # x - y + 128 >= 0
                    (-1, 0, [[1, 256]]),     # y - x >= 0
                    (-1, -128, [[1, 256]]),  # y - x - 128 >= 0
                ]
                for qp in range(NQB // 2):
                    ctx_hf = [
                        cxp.tile([128, D], F32R, tag="ctx", name=f"ctx_{qp}_{i}")
                        for i in range(2)
                    ]
                    for h2 in range(H // 2):
                        dc = h2
                        # scores for both heads back-to-back: the two heads
                        # sit in PE row groups 0-63 / 64-127 and co-run
                        psS_all = []
                        for hp_i in range(2):
                            hp = 64 * hp_i
                            psS = sc_ps.tile(
                                [128, 4, 256], F32, tag="sc",
                                name=f"psS_{qp}_{h2}_{hp_i}",
                            )
                            for kc in range(4):
                                kcol = qp * 256 + kc * 128
                                nc.tensor.matmul(
                                    psS[:, kc],
                                    krope[hp : hp + 64, dc, kcol : kcol + 128],
                                    qrope[hp : hp + 64, dc, qp * 256 : qp * 256 + 256],
                                    start=True, stop=True,
                                )
                            psS_all.append(psS)
                        pTs_h = []
                        for hp_i in range(2):
                            pT = pp.tile(
                                [128, 4, 256], BF16, tag="pT",
                                name=f"pT_{qp}_{h2}_{hp_i}",
                            )
                            nc.scalar.activation(
                                pT, psS_all[hp_i],
                                mybir.ActivationFunctionType.Exp, scale=0.125,
                            )
                            for j in range(4):
                                cm, base, pat = AFF[j]
                                nc.gpsimd.affine_select(
                                    out=pT[:, j], in_=pT[:, j],
                                    compare_op=mybir.AluOpType.is_ge, fill=0.0,
                                    base=base, channel_multiplier=cm, pattern=pat,
                                )
                            pTs_h.append(pT)
                        for hp_i in range(2):
                            h = 2 * h2 + hp_i
                            pT = pTs_h[hp_i]
                            for hf in range(2):  # query halves use chunks hf..hf+2
                                psC = ctx_ps.tile([128, VW], F32, tag="ctx")
                                for i, kc in enumerate(range(hf, hf + NKC)):
                                    nc.tensor.matmul(
                                        psC,
                                        pT[:, kc, hf * 128 : hf * 128 + 128],
                                        v_sb[:, qp * 2 + kc, h * VW : h * VW + VW],
                                        start=(i == 0), stop=(i == NKC - 1),
                                    )
                                qb = qp * 2 + hf
                                d2 = sm.tile([128, 1], F32, tag="d2")
                                rinv = sm.tile([128, 1], F32, tag="rinv")
                                nc.vector.tensor_tensor(
                                    out=d2, in0=psC[:, HD : HD + 1],
                                    in1=corr_sb[:, qb : qb + 1],
                                    op=mybir.AluOpType.subtract,
                                )
                                nc.vector.reciprocal(rinv, d2)
                                nc.vector.tensor_scalar_mul(
                                    ctx_hf[hf][:, h * HD : h * HD + HD], psC[:, 0:HD], rinv
                                )
                    # ctx -> ctxT (PE transpose) for both query blocks
                    for hf in range(2):
                        qb = qp * 2 + hf
                        for dc in range(DC):
                            psT = sc_ps.tile([128, 128], F32R, tag="sc", name=f"psT_{qp}_{hf}_{dc}")
                            nc.tensor.transpose(psT, ctx_hf[hf][:, ts(dc, 128)], ident)
                            if dc % 2 == 0:
                                nc.vector.tensor_copy(ctxT[:, dc, ts(qb, 128)], psT)
                            else:
                                nc.scalar.copy(out=ctxT[:, dc, ts(qb, 128)], in_=psT)

                # ---- Wo projection, split by query pair-block for overlap
                for qpo in range(2):
                    cs = slice(qpo * 256, qpo * 256 + 256)
                    for dco in range(DC):
                        ps = proj_ps.tile([128, 256], F32, tag="proj")
                        for k in range(DC):
                            nc.tensor.matmul(
                                ps, wo_at(k, dco), ctxT[:, k, cs],
                                start=(k == 0), stop=(k == DC - 1),
                            )
                        o_sb = op.tile([128, 256], F32, tag="o")
                        nc.scalar.activation(
                            o_sb, ps, mybir.ActivationFunctionType.Identity,
                            bias=bo_sb[:, dco : dco + 1], scale=1.0,
                        )
                        nc.sync.dma_start(out=outT[ts(dco, 128), cs], in_=o_sb)

            if loop_repeat is None:
                body()
            else:
                with tc.For_i(0, loop_repeat, 1):
                    body()

    nc.compile()
    return nc


_NC_CACHE = None


def _get_nc():
    global _NC_CACHE
    if _NC_CACHE is None:
        _NC_CACHE = build_nc()
    return _NC_CACHE


def _host_tables():
    """RoPE cos/sin tables, dim-major, tiled to 128 partitions (2 heads)."""
    inv_freq = 1.0 / (THETA ** (np.arange(0, HD, 2, dtype=np.float32) / HD))  # [32]
    ifq64 = np.concatenate([inv_freq, inv_freq])  # dim d uses inv_freq[d % 32]

    def tables(positions):
        ang = ifq64[:, None] * positions[None, :].astype(np.float32)  # [64, n]
        cos = np.cos(ang).astype(np.float32)
        sin = np.sin(ang).astype(np.float32)
        sin2 = np.concatenate([sin[:32], -sin[32:]], axis=0)  # sign flip 2nd half
        return np.tile(cos, (2, 1)), np.tile(sin2, (2, 1))

    return tables


def _dc_block(w):
    """[D, D] -> [128, DC*D] with per-partition layout [dc, k, c]."""
    return np.ascontiguousarray(
        np.asarray(w, dtype=np.float32)
        .reshape(DC, 128, DC, 128)
        .transpose(1, 2, 0, 3)
        .reshape(128, DC * D)
    )


def prep_in_maps(input_sequence, Wq, Wk, Wv, Wo, bo):
    x = np.asarray(input_sequence, dtype=np.float32)
    wq_r = round_fp32r(_dc_block(Wq))
    wk_r = round_fp32r(_dc_block(Wk))
    wv_r = round_fp32r(np.asarray(Wv))
    wo_r = round_fp32r(_dc_block(Wo))
    bo_t = np.asarray(bo, dtype=np.float32).reshape(DC, 128).T.copy()

    tables = _host_tables()
    in_maps = []
    for c in range(8):
        b, t = c // 4, c % 4
        start = t * TC
        lo = start - HALF_W
        xt = np.zeros((D, TH), dtype=np.float32)
        vs = max(0, lo)
        xt[:, vs - lo : TH] = x[b, vs : start + TC, :].T
        cosq_t, sinq2_t = tables(np.arange(start, start + TC))
        cosk_t, sink2_t = tables(np.arange(lo, start + TC))
        qpos = np.arange(start, start + TC)
        corr = np.maximum(0, HALF_W - qpos).astype(np.float32).reshape(NQB, 128).T.copy()
        in_maps.append(
            {
                "xt": round_fp32r(xt),
                "wq": wq_r, "wk": wk_r, "wv": wv_r, "wo": wo_r,
                "bo": bo_t,
                "cosq": cosq_t, "sinq2": sinq2_t,
                "cosk": cosk_t, "sink2": sink2_t,
                "corr": corr,
                "ident": np.eye(128, dtype=np.float32),
                "perm32": np.eye(128, dtype=np.float32)[
                    [p ^ 32 for p in range(128)]
                ].copy(),
            }
        )
    return in_maps


def kernel(input_sequence, Wq, Wk, Wv, Wo, bo):
    nc = _get_nc()
    in_maps = prep_in_maps(input_sequence, Wq, Wk, Wv, Wo, bo)
    res = run_bass_kernel_spmd(nc, in_maps, list(range(8)))
    out = np.empty((B, S, D), dtype=np.float32)
    for c in range(8):
        b, t = c // 4, c % 4
        out[b, t * TC : t * TC + TC, :] = res.results[c]["outT"].T
    return out



# revision 41
# speedup vs baseline: 1.1423x; 1.0730x over previous
"""Trainium2 Bass kernel for sliding-window multi-head attention.

Problem (nn_MultiHeadAttention_74285754352148):
  B=2, S=2048, D=1024, H=16, HD=64, WINDOW=512 (causal, j in [i-256, i]),
  RoPE theta=10000, out = softmax(mask(QK^T)/8) V @ Wo + bo.

Sharding: batch x sequence across 8 cores (core c: batch c//4, tokens
[512*(c%4), 512*(c%4)+512)). Each core recomputes K/V for a 256-token halo;
no collectives. Host pre-transposes X and pre-rounds matmul operands to
fp32r (8-bit exp / 11-bit mantissa -> full-rate PE).

Key structure per core (all layouts chosen to avoid on-chip transposes of
activations except ctx):
  qropeT[d, tok]  = RoPE(Wq^T @ X^T)   (dim-major)
  kropeT[d, tok]  = RoPE(Wk^T @ X^T)
  V[tok, d]       = X @ Wv             (token-major, +ones column per head)
  scoresT[k, q]   = kropeT^T-slices @ qropeT-slices  (keys on partitions)
  pT = exp(scoresT/8); band masks applied in-place via gpsimd affine_select
  ctx[q, 65]      = pT-chunks^T @ V_aug  (col 64 = softmax denominator)
  ctx normalized via fused tensor_scalar copy, PE-transposed to ctxT
  outT[d, tok]    = Wo^T-slices @ ctxT  (+bias), host transposes back
"""

import numpy as np

import concourse.bass as bass
import concourse.bacc as bacc
import concourse.mybir as mybir
from concourse.tile import TileContext
from concourse.bass import ts
from concourse.bass_utils import run_bass_kernel_spmd

F32 = mybir.dt.float32
F32R = mybir.dt.float32r
BF16 = mybir.dt.bfloat16

B, S, D = 2, 2048, 1024
H, HD = 16, 64
HALF_W = 256          # window // 2: query i attends keys [i-256, i]
TC = 512              # tokens per core
TH = TC + HALF_W      # tokens incl halo = 768
NQB = TC // 128       # query blocks per core = 4
NKC = 3               # key chunks per query block (384 = 3*128)
DC = D // 128         # 8 partition chunks of the model dim
NTC = TH // 128       # token chunks incl halo = 6
VW = HD + 1           # per-head V width incl ones column = 65
THETA = 10000.0


def round_fp32r(x: np.ndarray) -> np.ndarray:
    """Round-to-nearest (ties-to-even-ish) to fp32r: low 12 mantissa bits zero."""
    b = np.ascontiguousarray(x, dtype=np.float32).view(np.uint32)
    out = (b + np.uint32(0x7FF) + ((b >> np.uint32(12)) & np.uint32(1))) & np.uint32(
        0xFFFFF000
    )
    return out.view(np.float32)


SHIFT_MODE = "pe"  # pe | sync | scalar | none (timing experiment only)


def build_nc(loop_repeat=None):
    nc = bacc.Bacc(None, target_bir_lowering=False)

    xt = nc.dram_tensor("xt", [D, TH], F32R, kind="ExternalInput")
    wq = nc.dram_tensor("wq", [128, DC * D], F32R, kind="ExternalInput")
    wk = nc.dram_tensor("wk", [128, DC * D], F32R, kind="ExternalInput")
    wv = nc.dram_tensor("wv", [D, D], F32R, kind="ExternalInput")
    wo = nc.dram_tensor("wo", [128, DC * D], F32R, kind="ExternalInput")
    bo = nc.dram_tensor("bo", [128, DC], F32, kind="ExternalInput")
    cosq = nc.dram_tensor("cosq", [128, TC], F32, kind="ExternalInput")
    sinq2 = nc.dram_tensor("sinq2", [128, TC], F32, kind="ExternalInput")
    cosk = nc.dram_tensor("cosk", [128, TH], F32, kind="ExternalInput")
    sink2 = nc.dram_tensor("sink2", [128, TH], F32, kind="ExternalInput")
    corr = nc.dram_tensor("corr", [128, NQB], F32, kind="ExternalInput")
    ident_d = nc.dram_tensor("ident", [128, 128], F32R, kind="ExternalInput")
    perm_d = nc.dram_tensor("perm32", [128, 128], F32R, kind="ExternalInput")
    outT = nc.dram_tensor("outT", [D, TC], F32, kind="ExternalOutput")

    with TileContext(nc) as tc:
        with (
            tc.tile_pool(name="qkp", bufs=1) as qkp,
            tc.tile_pool(name="vp", bufs=1) as vp,
            tc.tile_pool(name="tbl", bufs=1) as tbl,
            tc.tile_pool(name="sm", bufs=8) as sm,
            tc.tile_pool(name="wpool", bufs=3) as wpool,
            tc.tile_pool(name="xtp", bufs=1) as xtp,
            tc.tile_pool(name="uwp", bufs=2) as uwp,
            tc.tile_pool(name="pp", bufs=3) as pp,
            tc.tile_pool(name="cxp", bufs=2) as cxp,
            tc.tile_pool(name="cxtp", bufs=1) as cxtp,
            tc.tile_pool(name="op", bufs=3) as op,
            tc.tile_pool(name="proj_ps", bufs=2, space="PSUM") as proj_ps,
            tc.tile_pool(name="sc_ps", bufs=2, space="PSUM") as sc_ps,
            tc.tile_pool(name="ctx_ps", bufs=2, space="PSUM") as ctx_ps,
        ):
            # ---- constant/table loads ----
            cosq_sb = tbl.tile([128, TC], F32)
            sinq2_sb = tbl.tile([128, TC], F32)
            cosk_sb = tbl.tile([128, TH], F32)
            sink2_sb = tbl.tile([128, TH], F32)
            corr_sb = tbl.tile([128, NQB], F32)
            bo_sb = tbl.tile([128, DC], F32)
            for t_dram, t_sb in [
                (cosq, cosq_sb),
                (sinq2, sinq2_sb),
                (cosk, cosk_sb),
                (sink2, sink2_sb),
                (corr, corr_sb),
                (bo, bo_sb),
            ]:
                nc.scalar.dma_start(out=t_sb, in_=t_dram[:, :])
            ident = tbl.tile([128, 128], F32R)
            nc.scalar.dma_start(out=ident, in_=ident_d[:, :])
            perm32 = tbl.tile([128, 128], F32R)
            nc.scalar.dma_start(out=perm32, in_=perm_d[:, :])
            ones16 = tbl.tile([128, H], F32)
            nc.vector.memset(ones16, 1.0)

            def body():
                # ---- input loads: query cols first so Q-proj starts early
                xt_sb = xtp.tile([128, DC, TH], F32R)
                for k in range(DC):
                    nc.gpsimd.dma_start(
                        out=xt_sb[:, k, HALF_W:TH], in_=xt[ts(k, 128), HALF_W:TH]
                    )

                def load_w_blocked(w_dram, nm):
                    """dc-blocked: host layout [p, dc, k, c]; access (k, dc)."""
                    halves = []
                    for hh in range(2):
                        w_sb = wpool.tile(
                            [128, DC // 2, DC, 128], F32R, tag="w", name=f"w_{nm}{hh}"
                        )
                        for j in range(2):
                            off = (hh * 4 + 2 * j) * D
                            nc.sync.dma_start(
                                out=w_sb[:, 2 * j : 2 * j + 2],
                                in_=w_dram[:, off : off + 2 * D],
                            )
                        halves.append(w_sb)
                    return lambda k, dc: halves[dc // 4][:, dc % 4, k]

                def load_w(w_dram, nm):
                    """Two half-matrix tiles [128, 4, 1024] sharing 3 slots."""
                    halves = []
                    for hh in range(2):
                        w_sb = wpool.tile(
                            [128, DC // 2, D], F32R, tag="w", name=f"w_{nm}{hh}"
                        )
                        for k in range(DC // 2):
                            nc.sync.dma_start(
                                out=w_sb[:, k], in_=w_dram[ts(hh * 4 + k, 128), :]
                            )
                        halves.append(w_sb)
                    return lambda k: halves[k // 4][:, k % 4]

                wq_at = load_w_blocked(wq, "q")
                for k in range(DC):
                    nc.gpsimd.dma_start(
                        out=xt_sb[:, k, 0:HALF_W], in_=xt[ts(k, 128), 0:HALF_W]
                    )
                wk_at = load_w_blocked(wk, "k")

                qrope = qkp.tile([128, DC, TC], F32R)
                krope = qkp.tile([128, DC, TH], F32R)

                def rope_epilogue(ps, cos_sb, sin2_sb, cslc, out_ap):
                    """out = ps*cos + shift32(ps*sin2); ps is PSUM [128, n]."""
                    n = ps.shape[-1]
                    u = uwp.tile([128, n], F32, tag="u")
                    nc.vector.scalar_tensor_tensor(
                        out=u, in0=ps, scalar=1.0, in1=cos_sb[:, cslc],
                        op0=mybir.AluOpType.bypass, op1=mybir.AluOpType.mult,
                    )
                    if SHIFT_MODE == "pe":
                        w = uwp.tile([128, n], F32R, tag="w")
                        nc.vector.scalar_tensor_tensor(
                            out=w, in0=ps, scalar=1.0, in1=sin2_sb[:, cslc],
                            op0=mybir.AluOpType.bypass, op1=mybir.AluOpType.mult,
                        )
                        ws_ps = sc_ps.tile([128, n], F32, tag="sc")
                        nc.tensor.matmul(ws_ps, perm32, w, start=True, stop=True)
                        nc.vector.tensor_add(out_ap, ws_ps, u)
                        return
                    w = uwp.tile([128, n], F32, tag="w")
                    ws = uwp.tile([128, n], F32, tag="ws")
                    nc.vector.scalar_tensor_tensor(
                        out=w, in0=ps, scalar=1.0, in1=sin2_sb[:, cslc],
                        op0=mybir.AluOpType.bypass, op1=mybir.AluOpType.mult,
                    )
                    if SHIFT_MODE == "none":
                        nc.vector.tensor_add(out_ap, u, w)
                        return
                    eng = nc.sync if SHIFT_MODE == "sync" else nc.scalar
                    for a in range(2):
                        eng.dma_start(out=ws[a * 64 : a * 64 + 32], in_=w[a * 64 + 32 : a * 64 + 64])
                        eng.dma_start(out=ws[a * 64 + 32 : a * 64 + 64], in_=w[a * 64 : a * 64 + 32])
                    nc.vector.tensor_add(out_ap, u, ws)

                # ---- Q^T projection + RoPE (dim-major) ----
                for dc in range(DC):
                    ps = proj_ps.tile([128, TC], F32, tag="proj")
                    for k in range(DC):
                        nc.tensor.matmul(
                            ps, wq_at(k, dc), xt_sb[:, k, HALF_W:TH],
                            start=(k == 0), stop=(k == DC - 1),
                        )
                    rope_epilogue(ps, cosq_sb, sinq2_sb, slice(0, TC), qrope[:, dc])

                # ---- K^T projection + RoPE, two 384-col halves ----
                for dc in range(DC):
                    for half in range(2):
                        cs = slice(half * 384, half * 384 + 384)
                        ps = proj_ps.tile([128, 384], F32, tag="proj")
                        for k in range(DC):
                            nc.tensor.matmul(
                                ps, wk_at(k, dc), xt_sb[:, k, cs],
                                start=(k == 0), stop=(k == DC - 1),
                            )
                        rope_epilogue(ps, cosk_sb, sink2_sb, cs, krope[:, dc, cs])

                wv_at = load_w(wv, "v")

                # ---- V projection (token-major, 65-wide per-head groups) ----
                v_sb = vp.tile([128, NTC, H * VW], BF16)
                for tcn in range(NTC):
                    v_grp = v_sb[:, tcn].rearrange("p (h c) -> p h c", c=VW)
                    nc.vector.tensor_copy(
                        v_grp[:, :, HD:VW], ones16.rearrange("p (a b) -> p a b", b=1)
                    )
                    for half in range(2):
                        ps = proj_ps.tile([128, 512], F32, tag="proj")
                        for k in range(DC):
                            nc.tensor.matmul(
                                ps, xt_sb[:, k, ts(tcn, 128)], wv_at(k)[:, ts(half, 512)],
                                start=(k == 0), stop=(k == DC - 1),
                            )
                        nc.scalar.copy(
                            out=v_grp[:, half * 8 : half * 8 + 8, 0:HD],
                            in_=ps.rearrange("p (h c) -> p h c", c=HD),
                        )

                wo_at = load_w_blocked(wo, "o")

                # ---- attention: per 256-query pair-block, per head ----
                # scoresT chunks kc=0..3 cover keys [qs-256, qs+256) for the
                # 256-query block at qs; each chunk needs exactly one affine
                # band mask (x = key partition, y = query free coord).
                ctxT = cxtp.tile([128, DC, TC], F32R)
                AFF = [
                    (1, 0, [[-1, 256]]),     # x - y >= 0
                    (1, 128, [[-1, 256]]),   # x - y + 128 >= 0
                    (-1, 0, [[1, 256]]),     # y - x >= 0
                    (-1, -128, [[1, 256]]),  # y - x - 128 >= 0
                ]
                for qp in range(NQB // 2):
                    ctx_hf = [
                        cxp.tile([128, D], F32R, tag="ctx", name=f"ctx_{qp}_{i}")
                        for i in range(2)
                    ]
                    for h2 in range(H // 2):
                        dc = h2
                        # scores for both heads back-to-back: the two heads
                        # sit in PE row groups 0-63 / 64-127 and co-run
                        psS_all = []
                        for hp_i in range(2):
                            hp = 64 * hp_i
                            psS = sc_ps.tile(
                                [128, 4, 256], F32, tag="sc",
                                name=f"psS_{qp}_{h2}_{hp_i}",
                            )
                            for kc in range(4):
                                kcol = qp * 256 + kc * 128
                                nc.tensor.matmul(
                                    psS[:, kc],
                                    krope[hp : hp + 64, dc, kcol : kcol + 128],
                                    qrope[hp : hp + 64, dc, qp * 256 : qp * 256 + 256],
                                    start=True, stop=True,
                                )
                            psS_all.append(psS)
                        pTs_h = []
                        for hp_i in range(2):
                            pT = pp.tile(
                                [128, 4, 256], BF16, tag="pT",
                                name=f"pT_{qp}_{h2}_{hp_i}",
                            )
                            nc.scalar.activation(
                                pT, psS_all[hp_i],
                                mybir.ActivationFunctionType.Exp, scale=0.125,
                            )
                            for j in range(4):
                                cm, base, pat = AFF[j]
                                nc.gpsimd.affine_select(
                                    out=pT[:, j], in_=pT[:, j],
                                    compare_op=mybir.AluOpType.is_ge, fill=0.0,
                                    base=base, channel_multiplier=cm, pattern=pat,
                                )
                            pTs_h.append(pT)
                        for hp_i in range(2):
                            h = 2 * h2 + hp_i
                            pT = pTs_h[hp_i]
                            for hf in range(2):  # query halves use chunks hf..hf+2
                                psC = ctx_ps.tile([128, VW], F32, tag="ctx")
                                for i, kc in enumerate(range(hf, hf + NKC)):
                                    nc.tensor.matmul(
                                        psC,
                                        pT[:, kc, hf * 128 : hf * 128 + 128],
                                        v_sb[:, qp * 2 + kc, h * VW : h * VW + VW],
                                        start=(i == 0), stop=(i == NKC - 1),
                                    )
                                qb = qp * 2 + hf
                                d2 = sm.tile([128, 1], F32, tag="d2")
                                rinv = sm.tile([128, 1], F32, tag="rinv")
                                nc.vector.tensor_tensor(
                                    out=d2, in0=psC[:, HD : HD + 1],
                                    in1=corr_sb[:, qb : qb + 1],
                                    op=mybir.AluOpType.subtract,
                                )
                                nc.vector.reciprocal(rinv, d2)
                                nc.vector.tensor_scalar_mul(
                                    ctx_hf[hf][:, h * HD : h * HD + HD], psC[:, 0:HD], rinv
                                )
                    # ctx -> ctxT (PE transpose) for both query blocks
                    for hf in range(2):
                        qb = qp * 2 + hf
                        for dc in range(DC):
                            psT = sc_ps.tile([128, 128], F32R, tag="sc", name=f"psT_{qp}_{hf}_{dc}")
                            nc.tensor.transpose(psT, ctx_hf[hf][:, ts(dc, 128)], ident)
                            if dc % 2 == 0:
                                nc.vector.tensor_copy(ctxT[:, dc, ts(qb, 128)], psT)
                            else:
                                nc.scalar.copy(out=ctxT[:, dc, ts(qb, 128)], in_=psT)

                # ---- Wo projection, split by query pair-block for overlap
                for qpo in range(2):
                    cs = slice(qpo * 256, qpo * 256 + 256)
                    for dco in range(DC):
                        ps = proj_ps.tile([128, 256], F32, tag="proj")
                        for k in range(DC):
                            nc.tensor.matmul(
                                ps, wo_at(k, dco), ctxT[:, k, cs],
                                start=(k == 0), stop=(k == DC - 1),
                            )
                        o_sb = op.tile([128, 256], F32, tag="o")
                        nc.scalar.activation(
                            o_sb, ps, mybir.ActivationFunctionType.Identity,
                            bias=bo_sb[:, dco : dco + 1], scale=1.0,
                        )
                        nc.sync.dma_start(out=outT[ts(dco, 128), cs], in_=o_sb)

            if loop_repeat is None:
                body()
            else:
                with tc.For_i(0, loop_repeat, 1):
                    body()

    nc.compile()
    return nc


_NC_CACHE = None


def _get_nc():
    global _NC_CACHE
    if _NC_CACHE is None:
        _NC_CACHE = build_nc()
    return _NC_CACHE


def _host_tables():
    """RoPE cos/sin tables, dim-major, tiled to 128 partitions (2 heads)."""
    inv_freq = 1.0 / (THETA ** (np.arange(0, HD, 2, dtype=np.float32) / HD))  # [32]
    ifq64 = np.concatenate([inv_freq, inv_freq])  # dim d uses inv_freq[d % 32]

    def tables(positions):
        ang = ifq64[:, None] * positions[None, :].astype(np.float32)  # [64, n]
        cos = np.cos(ang).astype(np.float32)
        sin = np.sin(ang).astype(np.float32)
        sin2 = np.concatenate([sin[:32], -sin[32:]], axis=0)  # sign flip 2nd half
        return np.tile(cos, (2, 1)), np.tile(sin2, (2, 1))

    return tables


def _dc_block(w):
    """[D, D] -> [128, DC*D] with per-partition layout [dc, k, c]."""
    return np.ascontiguousarray(
        np.asarray(w, dtype=np.float32)
        .reshape(DC, 128, DC, 128)
        .transpose(1, 2, 0, 3)
        .reshape(128, DC * D)
    )


def prep_in_maps(input_sequence, Wq, Wk, Wv, Wo, bo):
    x = np.asarray(input_sequence, dtype=np.float32)
    wq_r = round_fp32r(_dc_block(Wq))
    wk_r = round_fp32r(_dc_block(Wk))
    wv_r = round_fp32r(np.asarray(Wv))
    wo_r = round_fp32r(_dc_block(Wo))
    bo_t = np.asarray(bo, dtype=np.float32).reshape(DC, 128).T.copy()

    tables = _host_tables()
    in_maps = []
    for c in range(8):
        b, t = c // 4, c % 4
        start = t * TC
        lo = start - HALF_W
        xt = np.zeros((D, TH), dtype=np.float32)
        vs = max(0, lo)
        xt[:, vs - lo : TH] = x[b, vs : start + TC, :].T
        cosq_t, sinq2_t = tables(np.arange(start, start + TC))
        cosk_t, sink2_t = tables(np.arange(lo, start + TC))
        qpos = np.arange(start, start + TC)
        corr = np.maximum(0, HALF_W - qpos).astype(np.float32).reshape(NQB, 128).T.copy()
        in_maps.append(
            {
                "xt": round_fp32r(xt),
                "wq": wq_r, "wk": wk_r, "wv": wv_r, "wo": wo_r,
                "bo": bo_t,
                "cosq": cosq_t, "sinq2": sinq2_t,
                "cosk": cosk_t, "sink2": sink2_t,
                "corr": corr,
                "ident": np.eye(128, dtype=np.float32),
                "perm32": np.eye(128, dtype=np.float32)[
                    [p ^ 32 for p in range(128)]
                ].copy(),
            }
        )
    return in_maps


def kernel(input_sequence, Wq, Wk, Wv, Wo, bo):
    nc = _get_nc()
    in_maps = prep_in_maps(input_sequence, Wq, Wk, Wv, Wo, bo)
    res = run_bass_kernel_spmd(nc, in_maps, list(range(8)))
    out = np.empty((B, S, D), dtype=np.float32)
    for c in range(8):
        b, t = c // 4, c % 4
        out[b, t * TC : t * TC + TC, :] = res.results[c]["outT"].T
    return out

